# revision 58
# baseline (speedup 1.0000x reference)
"""Trainium2 Bass kernel for nn_BipartiteGraphConvolution_63874753626723.

Computation (see reference):
    norm = ||edge_weight||_2
    conv[r] = sum_e (edge_weight[e]/norm) * left_features[col[e]]   (row[e]==r)
    out = (right_features + temp[1] * (c - conv)) * SCALE

The edge list is structured: edge e = r*12+k has row=r, col=(13r+k) % M, so
dest row r consumes L rows 13r..13r+11.  Each of 8 cores handles 12500 dest
rows (padded to 12544 = 14*7*128).

This version computes in bf16 with a TensorEngine segment-reduction:
  - The host pre-permutes left_features into a "PE layout"
    L_pe[q=(pp,k) padded to 128, s, gc=cb*7+g, d], DMA'd in 2-supertile
    ~3MB chunks on all 128 partitions (measured: 120-partition transfers
    run ~1.8x slower and ~1.5MB transfers well below line rate).
  - DVE does one bf16 2x-mode multiply msg = L * w per supertile; the
    broadcast of w along d keeps 2x mode via host-duplicated (w,w) pairs
    (innermost AP step 1, count 2 — a stride-0 operand would drop to 1x).
  - TensorE reduces over the edge slots with 13 constant 0/1 selection
    matmuls (contraction q=(pp,k), output rows i=cb*10+pp) into PSUM.
  - The edge-weight norm: per-core partial sum of squares (ACT Square with
    accumulate) + AllReduce across the 8 cores; folded into the ACT
    PSUM->SBUF copy as a per-partition scale.  7 PSUM banks of runway keep
    the main pipeline running while the collective is in flight.
  - right/c/out are bf16; the host converts the output back to fp32.
A numpy fallback covers inputs whose edge_index is not the structured
pattern.
"""

import os
import sys

if "/opt/trn_rl_repo" in sys.path:
    sys.path.remove("/opt/trn_rl_repo")

import numpy as np

N = 100000
M = 100000
DEG = 12
D = 64
E = N * DEG
SCALE = 0.4251202479144762

NCORES = 8
RPC = N // NCORES            # real dest rows per core: 12500
P = 128
S = 14                       # supertiles per core
G = 7                        # 128-row groups per supertile
RP = S * G * P               # padded dest rows per core: 12544
TT = 10                      # dest rows per cb-block
CB = 13                      # cb-blocks (ceil(128/10))
KP = TT * DEG                # used partitions in the PE layout: 120
GC = CB * G                  # (cb,g) pairs per supertile: 91
SGC = S * GC                 # 1274
FD = GC * D                  # free elems per supertile: 5824

L8 = bool(int(os.environ.get("BGC_L8", "0")))   # fp8 L (direct, 1x multiply)
NORM_MODE = os.environ.get("BGC_NORM", "cc")    # "cc" (AllReduce) or "full"
SKIP = set(os.environ.get("BGC_SKIP", "").split(","))  # mult,mm,epi bisect
EPI = os.environ.get("BGC_EPI", "inline")  # inline | late
CS = int(os.environ.get("BGC_CS", "2"))    # supertiles per L-DMA chunk
CHUNKS = []                  # (s0, n_supertiles) per L-DMA chunk
_s = 0
while _s < S:
    CHUNKS.append((_s, min(CS, S - _s)))
    _s += CS
NCHK = len(CHUNKS)
GD = int(os.environ.get("BGC_GD", "62"))   # fp8: gc columns on DVE (rest GP)
OC = int(os.environ.get("BGC_OC", "1"))    # L-chunks per out-DMA group

_PROG = None
_IDX = None   # cached host-side gather indices


def _build_program():
    import concourse.bacc as bacc
    import concourse.tile as tile
    import concourse.mybir as mybir
    from contextlib import ExitStack

    f32 = mybir.dt.float32
    bf16 = mybir.dt.bfloat16
    f8 = mybir.dt.float8e4
    nc = bacc.Bacc("TRN2", target_bir_lowering=False, debug=False,
                   num_devices=NCORES)

    lpe = nc.dram_tensor("lpe", [NCHK * P, CS * FD], f8 if L8 else bf16,
                         kind="ExternalInput")
    w2d = nc.dram_tensor("w2d", [KP, SGC * 2], bf16, kind="ExternalInput")
    rsl = nc.dram_tensor("rsl", [P, S * G * D], bf16, kind="ExternalInput")
    csl = nc.dram_tensor("csl", [P, S * G * 2], bf16, kind="ExternalInput")
    # tb[:,0] = -SCALE*temp1, tb[:,1] = +SCALE*temp1 (host-prescaled)
    tb = nc.dram_tensor("tb", [P, 2], f32, kind="ExternalInput")
    lhs = nc.dram_tensor("lhs", [KP, CB * P], f8 if L8 else bf16,
                         kind="ExternalInput")
    ewf = None
    if NORM_MODE == "full":
        ewf = nc.dram_tensor("ewf", [E], bf16, kind="ExternalInput")
    out = nc.dram_tensor("out", [P, S * G * D], bf16, kind="ExternalOutput")

    reps = int(os.environ.get("BGC_REPS", "1"))
    with tile.TileContext(nc) as tc, ExitStack() as ctx:
        if reps > 1:
            with tc.For_i(0, reps, 1):
                _kernel_body(ctx, tc, mybir, lpe, w2d, rsl, csl, tb, lhs,
                             ewf, out)
        else:
            _kernel_body(ctx, tc, mybir, lpe, w2d, rsl, csl, tb, lhs, ewf,
                         out)

    nc.compile()
    return nc


def _kernel_body(ctx, tc, mybir, lpe, w2d, rsl, csl, tb, lhs, ewf, out):
    f32 = mybir.dt.float32
    bf16 = mybir.dt.bfloat16
    Alu = mybir.AluOpType
    Act = mybir.ActivationFunctionType
    nc = tc.nc

    const_pool = ctx.enter_context(tc.tile_pool(name="const", bufs=1))
    sc_pool = ctx.enter_context(tc.tile_pool(name="sc", bufs=1))
    wc_pool = ctx.enter_context(tc.tile_pool(name="wc", bufs=1))
    lpool = ctx.enter_context(tc.tile_pool(name="l", bufs=int(os.environ.get("BGC_LB", "3"))))
    rpool = ctx.enter_context(tc.tile_pool(name="r", bufs=4))
    obufs = 4 if EPI == "inline" else S
    opool = ctx.enter_context(tc.tile_pool(name="o", bufs=obufs))
    apool = ctx.enter_context(tc.tile_pool(name="a", bufs=obufs))
    psum_pool = ctx.enter_context(tc.tile_pool(name="ps", bufs=7,
                                               space="PSUM"))
    psn_pool = ctx.enter_context(tc.tile_pool(name="psn", bufs=1,
                                              space="PSUM"))

    # ---- persistent loads -------------------------------------------------
    ldt = mybir.dt.float8e4 if L8 else bf16
    lhs_sb = const_pool.tile([KP, CB, P], ldt)
    nc.sync.dma_start(lhs_sb[:], lhs.ap().rearrange("q (c i) -> q c i", c=CB))
    wfull = wc_pool.tile([KP, SGC * 2], bf16)
    nc.sync.dma_start(wfull[:], w2d.ap())
    cfull = wc_pool.tile([P, S * G * 2], bf16)
    nc.sync.dma_start(cfull[:], csl.ap())
    tbt = sc_pool.tile([P, 2], f32)
    nc.sync.dma_start(tbt[:], tb.ap())

    # ---- norm -------------------------------------------------------------
    stot = sc_pool.tile([P, 1], f32)
    ones = const_pool.tile([P, P], f32)
    nc.vector.memset(ones[:], 1.0)
    if NORM_MODE == "cc":
        # partial sumsq over this core's (duplicated) weights, then
        # AllReduce; duplication folded into the final sqrt(0.5*x).
        # Square must NOT be in-place: the main loop reads wfull.
        spw = sc_pool.tile([KP, 1], f32)
        wsq = sc_pool.tile([KP, SGC * 2], f32, tag="wsq")
        nc.scalar.activation(wsq[:], wfull[:], Act.Square, accum_out=spw[:])
        psP = psn_pool.tile([P, 1], f32, tag="psnorm")
        nc.tensor.matmul(psP[:], ones[:][0:KP, :], spw[:], start=True,
                         stop=True)
        part_sb = sc_pool.tile([P, 1], f32)
        nc.scalar.activation(part_sb[:], psP[:], Act.Copy)
        ccdram = ctx.enter_context(tc.tile_pool(name="ccdram", bufs=1,
                                                space="DRAM"))
        ib = ccdram.tile([P, 1], f32)
        ob = ccdram.tile([P, 1], f32)
        nc.gpsimd.dma_start(ib[:], part_sb[:])
        nc.gpsimd.collective_compute(
            "AllReduce", Alu.add, replica_groups=[list(range(NCORES))],
            ins=[ib[:].opt()], outs=[ob[:].opt()])
        nc.gpsimd.dma_start(stot[:], ob[:])
        inv_scale = 0.5  # duplicated (w,w) pairs double the sumsq
    else:
        NCH = 5
        EWPP = E // P
        CHW = EWPP // NCH
        ewv = ewf.ap().rearrange("(p f) -> p f", p=P)
        sp = sc_pool.tile([P, NCH], f32)
        ew_pool = ctx.enter_context(tc.tile_pool(name="ew", bufs=3))
        sqd = sc_pool.tile([P, CHW], f32, tag="sqd")
        for j in range(NCH):
            ewt = ew_pool.tile([P, CHW], bf16)
            nc.scalar.dma_start(ewt[:], ewv[:, j * CHW:(j + 1) * CHW])
            nc.scalar.activation(sqd[:], ewt[:], Act.Square,
                                 accum_out=sp[:, j:j + 1])
        psS = psn_pool.tile([P, NCH], f32, tag="psnorm")
        nc.tensor.matmul(psS[:], ones[:], sp[:], start=True, stop=True)
        # free-dim sum over the NCH partials via ACT accumulate (keeps the
        # norm chain off the DVE queue)
        scr = sc_pool.tile([P, NCH], f32, tag="scr")
        nc.scalar.activation(scr[:], psS[:], Act.Copy, accum_out=stot[:])
        inv_scale = 1.0

    # ctSd[p, s, g, 2] = SCALE*temp1 * c (duplicated pairs) — norm-free
    ctSd = wc_pool.tile([P, S * G * 2], bf16)
    nc.scalar.activation(ctSd[:], cfull[:], Act.Copy, scale=tbt[:, 1:2])

    negs = sc_pool.tile([P, 1], f32)
    if EPI == "inline":
        # norm tail up-front: safe when the norm is ready early (full mode,
        # or a fast collective) — epilogues then need no separate pass B.
        normt = sc_pool.tile([P, 1], f32)
        nc.scalar.activation(normt[:], stot[:], Act.Sqrt, scale=inv_scale)
        inv = sc_pool.tile([P, 1], f32)
        nc.vector.reciprocal(inv[:], normt[:])
        nc.scalar.activation(negs[:], inv[:], Act.Copy, scale=tbt[:, 0:1])

    # ---- main loop --------------------------------------------------------
    lv = lpe.ap()        # [KP, S*FD]
    rv = rsl.ap()        # [P, S*G*D]
    ov = out.ap()        # [P, S*G*D]

    # Pass A: everything that does not need the norm.  Pass B (after all of
    # A in program order) folds in negs and stores — so the FIFO engine
    # queues never block main-pipeline work behind the AllReduce.
    # L comes in CS-supertile chunks on all 128 partitions: both are big
    # DMA-efficiency wins (120-partition transfers run ~1.8x slower, and
    # ~3MB transfers reach line rate where ~1.5MB ones do not).
    pending = []
    for ch, (s0, nst) in enumerate(CHUNKS):
        Lt = lpool.tile([P, CS * FD], ldt)
        nc.sync.dma_start(Lt[:, 0:nst * FD],
                          lv[ch * P:(ch + 1) * P, 0:nst * FD])
        Rt = rpool.tile([P, CS * G * D], bf16)
        nc.sync.dma_start(Rt[:, 0:nst * G * D],
                          rv[:, s0 * G * D:(s0 + nst) * G * D])
        if EPI == "inline" and ch % OC == 0:
            oc0 = ch
            Oc = opool.tile([P, OC * CS * G * D], bf16)

        for st2 in range(nst):
            s = s0 + st2
            Ls = Lt[:, st2 * FD:(st2 + 1) * FD]
            if "mult" not in SKIP and not L8:
                # msg = L * w (w broadcast along d at 2x via (w,w) pair view)
                lq = Ls[0:KP].rearrange("q (gc dh two) -> q gc dh two",
                                        gc=GC, two=2)
                wq = wfull[:, s * GC * 2:(s + 1) * GC * 2] \
                    .rearrange("q (gc two) -> q gc two", two=2) \
                    .unsqueeze(2).to_broadcast([KP, GC, D // 2, 2])
                nc.vector.tensor_tensor(lq, lq, wq, op=Alu.mult)
            elif "mult" not in SKIP:
                # fp8 runs DVE at 1x — split the multiply with GPSIMD
                lq = Ls[0:KP].rearrange("q (gc d) -> q gc d", gc=GC)
                wv = wfull[:, s * GC * 2:(s + 1) * GC * 2] \
                    .rearrange("q (gc two) -> q gc two", two=2)
                if GD > 0:
                    wq = wv[:, 0:GD, 0:1].to_broadcast([KP, GD, D])
                    nc.vector.tensor_tensor(lq[:, 0:GD, :], lq[:, 0:GD, :],
                                            wq, op=Alu.mult)
                if GD < GC:
                    wg = wv[:, GD:GC, 0:1].to_broadcast([KP, GC - GD, D])
                    nc.gpsimd.tensor_tensor(lq[:, GD:GC, :],
                                            lq[:, GD:GC, :], wg,
                                            op=Alu.mult)

            if "mm" not in SKIP:
                # PE segment reduction: acc[i=(cb*10+pp),(g,d)] += sel . msg
                # (lhs rows 120-127 are zero, so the 8 pad partitions of Ls
                # never contribute)
                acc = psum_pool.tile([P, G * D], f32)
                for cb in range(CB):
                    nc.tensor.matmul(acc[:],
                                     lhs_sb[:, cb, :],
                                     Ls[0:KP, cb * G * D:(cb + 1) * G * D],
                                     start=(cb == 0), stop=(cb == CB - 1))

            if "mm" in SKIP or "epi" in SKIP:
                if "epi" not in SKIP:
                    Ot = opool.tile([P, G * D], bf16)
                    nc.vector.tensor_scalar(
                        Ot[:], Rt[:, st2 * G * D:(st2 + 1) * G * D], SCALE,
                        None, op0=Alu.mult)
                    oeng = (nc.gpsimd if os.environ.get("BGC_OQ") == "gp"
                            else nc.scalar)
                    oeng.dma_start(ov[:, s * G * D:(s + 1) * G * D],
                                   Ot[:])
                continue

            if EPI == "inline":
                # negs folded into the PSUM->SBUF copy; whole epilogue here
                acc_sb = apool.tile([P, G * D], bf16)
                nc.scalar.activation(acc_sb[:], acc[:], Act.Copy,
                                     scale=negs[:])
                so = (s - CHUNKS[oc0][0])
                Ot = Oc[:, so * G * D:(so + 1) * G * D]
                nc.vector.tensor_scalar(
                    Ot, Rt[:, st2 * G * D:(st2 + 1) * G * D], SCALE, None,
                    op0=Alu.mult)
                oq = Ot.rearrange("p (g dh two) -> p g dh two", g=G, two=2)
                cq = ctSd[:, s * G * 2:(s + 1) * G * 2] \
                    .rearrange("p (g two) -> p g two", two=2) \
                    .unsqueeze(2).to_broadcast([P, G, D // 2, 2])
                nc.vector.tensor_tensor(oq, oq, cq, op=Alu.add)
                nc.vector.tensor_tensor(Ot, Ot, acc_sb[:], op=Alu.add)
                continue

            # unscaled PSUM->SBUF copy frees the bank, no norm dependency
            acc_sb = apool.tile([P, G * D], bf16)
            nc.scalar.activation(acc_sb[:], acc[:], Act.Copy)
            # Ot = SCALE*right + pscale*c  (STT is verifier-limited to 3D
            # APs, so 2D tensor_scalar at 4x + 4D pair-view TT add at 2x)
            Ot = opool.tile([P, G * D], bf16)
            nc.vector.tensor_scalar(
                Ot[:], Rt[:, st2 * G * D:(st2 + 1) * G * D], SCALE, None,
                op0=Alu.mult)
            oq = Ot[:].rearrange("p (g dh two) -> p g dh two", g=G, two=2)
            cq = ctSd[:, s * G * 2:(s + 1) * G * 2] \
                .rearrange("p (g two) -> p g two", two=2) \
                .unsqueeze(2).to_broadcast([P, G, D // 2, 2])
            nc.vector.tensor_tensor(oq, oq, cq, op=Alu.add)
            pending.append((s, Ot, acc_sb))

        last_of_group = (ch % OC == OC - 1) or (ch == NCHK - 1)
        if (EPI == "inline" and "mm" not in SKIP and "epi" not in SKIP
                and last_of_group):
            og0 = CHUNKS[oc0][0]
            ntot = s0 + nst - og0
            oeng = (nc.scalar if os.environ.get("BGC_OQ") == "act"
                    else nc.gpsimd)
            oeng.dma_start(
                ov[:, og0 * G * D:(og0 + ntot) * G * D],
                Oc[:, 0:ntot * G * D])

    # negs = -SCALE*temp1/norm and pass B.  tile_wait_until pins their
    # scheduler dispatch time after all main-loop work, so the greedy
    # scheduler cannot wedge norm-dependent ops into the engine FIFOs ahead
    # of main-pipeline work (where they would block on the collective).
    # Out-DMAs go through the gpsimd SWDGE queue, whose only other work is
    # the collective chain itself.
    if EPI != "inline":
        waitb = float(os.environ.get("BGC_WAITB", "0.07"))
        with tc.tile_wait_until(waitb):
            normt = sc_pool.tile([P, 1], f32)
            nc.scalar.activation(normt[:], stot[:], Act.Sqrt,
                                 scale=inv_scale)
            inv = sc_pool.tile([P, 1], f32)
            nc.vector.reciprocal(inv[:], normt[:])
            nc.scalar.activation(negs[:], inv[:], Act.Copy,
                                 scale=tbt[:, 0:1])

            for s, Ot, acc_sb in pending:
                nc.vector.scalar_tensor_tensor(Ot[:], acc_sb[:], negs[:],
                                               Ot[:], op0=Alu.mult,
                                               op1=Alu.add)
                nc.gpsimd.dma_start(ov[:, s * G * D:(s + 1) * G * D], Ot[:])


# ---------------- host side ------------------------------------------------

def _build_lhs():
    lhsm = np.zeros((KP, CB, P), np.float32)
    for cb in range(CB):
        for pp in range(TT):
            i = cb * TT + pp
            if i < P:
                for k in range(DEG):
                    lhsm[pp * DEG + k, cb, i] = 1.0
    return lhsm.reshape(KP, CB * P)


def _indices():
    """Cached per-core gather indices for the PE layout."""
    global _IDX
    if _IDX is not None:
        return _IDX
    pp = np.arange(TT)
    k = np.arange(DEG)
    s = np.arange(S)
    cb = np.arange(CB)
    g = np.arange(G)
    i = cb[:, None] * TT + pp[None, :]                     # [CB, TT]
    valid = i < P
    iw = np.where(valid, i, 0)
    # u[s, cb, g, pp] : core-local dest row
    u = (s[:, None, None, None] * (G * P)
         + g[None, None, :, None] * P
         + iw[None, :, None, :])                           # [S, CB, G, TT]
    per_core = []
    for core in range(NCORES):
        r0 = core * RPC
        lrows = (13 * (r0 + u[..., None]) + k) % M         # [S,CB,G,TT,DEG]
        widx = 12 * (r0 + u[..., None]) + k                # [S,CB,G,TT,DEG]
        wmask = (valid[None, :, None, :, None]
                 & (u[..., None] < RPC))                   # [S,CB,G,TT,DEG]
        per_core.append((lrows, widx, wmask))
    _IDX = per_core
    return per_core


def _prep_in_maps(left_features, edge_weight, right_features, c, temp):
    import ml_dtypes
    bf16 = ml_dtypes.bfloat16
    ldt = ml_dtypes.float8_e4m3 if L8 else bf16

    wpad = np.zeros(12 * (RPC * (NCORES - 1) + RP), np.float32)
    wpad[:E] = edge_weight
    rpad = np.zeros((RPC * (NCORES - 1) + RP, D), np.float32)
    rpad[:N] = right_features
    cpad = np.zeros(RPC * (NCORES - 1) + RP, np.float32)
    cpad[:N] = c[:, 0]
    t1 = np.float32(temp[1])
    tbv = np.broadcast_to(
        np.array([-SCALE * t1, SCALE * t1], np.float32), (P, 2)).copy()
    lhsm = _build_lhs().astype(ldt if L8 else bf16)

    if L8:
        lsrc = np.clip(left_features, -240.0, 240.0).astype(ldt)
    else:
        lsrc = left_features.astype(bf16)

    in_maps = []
    for core, (lrows, widx, wmask) in enumerate(_indices()):
        r0 = core * RPC
        # L_pe[ch, q=(pp,k) padded to 128, st2, gc=cb*G+g, d]
        lpe = lsrc[lrows]                                  # [S,CB,G,TT,DEG,D]
        lpe = lpe.transpose(3, 4, 0, 1, 2, 5).reshape(KP, S, FD)
        lpad = np.zeros((P, S, FD), lpe.dtype)
        lpad[:KP] = lpe
        lout = np.zeros((NCHK, P, CS * FD), lpe.dtype)
        for ch, (s0, nst) in enumerate(CHUNKS):
            lout[ch, :, 0:nst * FD] = \
                lpad[:, s0:s0 + nst].reshape(P, nst * FD)
        lpe = lout.reshape(NCHK * P, CS * FD)
        # w2 duplicated pairs, zeroed on pad slots
        w2 = np.where(wmask, wpad[widx], 0.0)              # [S,CB,G,TT,DEG]
        w2 = w2.transpose(3, 4, 0, 1, 2).reshape(KP, SGC)
        w2d = np.repeat(w2, 2, axis=1).astype(bf16)
        # right / c in [p, s, g, .] layout
        rs = rpad[r0:r0 + RP].reshape(S, G, P, D)
        rs = rs.transpose(2, 0, 1, 3).reshape(P, S * G * D).astype(bf16)
        cs = cpad[r0:r0 + RP].reshape(S, G, P).transpose(2, 0, 1)
        cs = np.repeat(cs.reshape(P, S * G), 2, axis=1).astype(bf16)
        im = {
            "lpe": np.ascontiguousarray(lpe),
            "w2d": np.ascontiguousarray(w2d),
            "rsl": np.ascontiguousarray(rs),
            "csl": np.ascontiguousarray(cs),
            "tb": tbv,
            "lhs": lhsm,
        }
        if NORM_MODE == "full":
            im["ewf"] = edge_weight.astype(bf16)
        in_maps.append(im)
    return in_maps


def _get_program():
    global _PROG
    if _PROG is None:
        _PROG = _build_program()
    return _PROG


def _structured(edge_index):
    ei = np.asarray(edge_index)
    if ei.shape != (E, 2):
        return False
    r = ei[:, 0].reshape(N, DEG)
    cc = ei[:, 1].reshape(N, DEG)
    rows = np.arange(N, dtype=np.int64)[:, None]
    offs = np.arange(DEG, dtype=np.int64)[None, :]
    return bool((r == rows).all() and (cc == (rows * 13 + offs) % M).all())


def _fallback(left_features, edge_index, edge_weight, right_features, c, temp):
    ei = np.asarray(edge_index)
    ew = np.asarray(edge_weight, dtype=np.float32)
    norm = np.float32(np.sqrt(np.sum(ew.astype(np.float64) ** 2)))
    w = ew / norm
    msg = left_features[ei[:, 1]] * w[:, None]
    conv = np.zeros((c.shape[0], left_features.shape[1]), np.float32)
    np.add.at(conv, ei[:, 0], msg)
    return ((right_features + temp[1] * (c - conv)) * np.float32(SCALE)).astype(
        np.float32)


def kernel(left_features, right_features_k, edge_index, edge_weight,
           right_features, c, b, temp):
    left_features = np.ascontiguousarray(left_features, dtype=np.float32)
    edge_weight = np.ascontiguousarray(edge_weight, dtype=np.float32)
    right_features = np.ascontiguousarray(right_features, dtype=np.float32)
    c = np.ascontiguousarray(c, dtype=np.float32)
    temp = np.asarray(temp, dtype=np.float32)

    if not _structured(edge_index):
        return _fallback(left_features, edge_index, edge_weight,
                         right_features, c, temp)

    from concourse import bass_utils

    nc = _get_program()
    in_maps = _prep_in_maps(left_features, edge_weight, right_features, c,
                            temp)
    res = bass_utils.run_bass_kernel_spmd(nc, in_maps, list(range(NCORES)))
    outp = np.empty((N, D), np.float32)
    for core in range(NCORES):
        o = np.asarray(res.results[core]["out"]).astype(np.float32)
        o = o.reshape(P, S, G, D).transpose(1, 2, 0, 3).reshape(RP, D)
        outp[core * RPC:(core + 1) * RPC] = o[:RPC]
    return outp
